# revision 28
# baseline (speedup 1.0000x reference)
"""Trainium2 Bass kernel for MHA block (LN -> QKV -> qk-LN -> RoPE -> masked attn -> out-proj).

Self-contained: hardcodes shapes B=2, L=2048, D=1024, H=16, Dh=64; runs on 8 NeuronCores
via bass_utils.run_bass_kernel_spmd. Sharding: core c = (batch b = c//4, head-group
g = c%4 of 4 heads). Weight columns are sliced per core so "our" 4 heads are always
columns 0:256 -> the device program is identical on all cores (SPMD).

Key structure (v2):
- bf16 compute throughout (weights, h, q/k/v, probs, ctx, out partials); fp32 PSUM.
- Sparse attention: seq_id is sorted per batch -> the mask is block diagonal. The
  kernel is compiled per seq-segment structure (computed from the actual input in
  kernel()); (q-block, k-tile) pairs with no segment overlap in EITHER batch are
  skipped entirely (scores, exp and PV). Masks rows (5 extra contraction rows at
  64:69 of qT/kT) give exact masking at segment boundaries.
- qk-LN stats (over the full 1024 dims) come from per-core partial sums gathered
  free on the ACT copy/square accumulators, with two 4-core-group AllReduces.
  Attention q-blocks are ordered so blocks only needing k-tiles < SP run while the
  second AllReduce is in flight.
- rstd = exp(-0.5*ln(var+eps)): Ln/Exp/Identity/Copy/Square all live in ONE ACT
  table -> no table-swap stalls between LN work and softmax exp.
- RoPE applied to raw q/k (linear), LN affine folded in afterwards:
  rot(LN(q)) = rstd*rot(q) + (-rstd*mu)*rot(ones); rope/fold are flat 2D bf16 ops
  with host-replicated tables.
"""

import numpy as np
import ml_dtypes
from contextlib import ExitStack

import concourse.bass as bass
import concourse.tile as tile
from concourse import bacc, mybir
from concourse import bass_utils

F32 = mybir.dt.float32
F32R = mybir.dt.float32r
BF16 = mybir.dt.bfloat16
AF = mybir.ActivationFunctionType
ALU = mybir.AluOpType

B, L, D = 2, 2048, 1024
H, DH = 16, 64
HPC = 4          # heads per core
CD = HPC * DH    # ctx dims per core = 256
P = 128
TT = L // P      # 16 token tiles
KC = D // P      # 8 contraction chunks
QB = 256         # query block width
NQB = L // QB    # 8 query blocks
EPS = 1e-5
ROPE_BASE = 10000.0
MASK_A = 8.0     # mask row scale; mask bias = -MASK_A^2 = -64 for masked pairs
KROWS = DH + 5   # contraction rows for scores: 64 dims + 5 mask rows
VB = DH + 1      # v block width (64 dims + ones col)
RG = [[0, 1, 2, 3], [4, 5, 6, 7]]


def build_bass(use_ln1b, use_qlw, use_klw, kts_per_qb):
    # three stats-AllReduce chunks, pipelined so no finalize waits long
    CH = [(0, 6), (6, 11), (11, 16)]
    nc = bacc.Bacc("TRN2", target_bir_lowering=False, debug=False, num_devices=8)

    # ---- DRAM I/O ----
    # weights/tables arrive host-prepacked partition-major ([128, contiguous])
    # so every DMA is 128 simple full lines: fast descriptor issue + bandwidth
    x_d = nc.dram_tensor("x", [L, D], F32, kind="ExternalInput").ap()
    wqk_d = nc.dram_tensor("wqk", [P, KC * 512], BF16, kind="ExternalInput").ap()
    wv_d = nc.dram_tensor("wv", [P, KC * CD], BF16, kind="ExternalInput").ap()
    wo_d = nc.dram_tensor("wo", [P, 2 * D], BF16, kind="ExternalInput").ap()
    mq_d = nc.dram_tensor("maskq", [5, L], BF16, kind="ExternalInput").ap()
    mk_d = nc.dram_tensor("maskk", [5, L], BF16, kind="ExternalInput").ap()
    cos4_d = nc.dram_tensor("cos4", [P, TT * CD], BF16, kind="ExternalInput").ap()
    sl4_d = nc.dram_tensor("sl4", [P, TT * CD // 2], BF16, kind="ExternalInput").ap()
    sh4_d = nc.dram_tensor("sh4", [P, TT * CD // 2], BF16, kind="ExternalInput").ap()
    r14_d = nc.dram_tensor("r14", [P, TT * CD], BF16, kind="ExternalInput").ap()
    idb_d = nc.dram_tensor("identb", [P, P], BF16, kind="ExternalInput").ap()
    on64_d = nc.dram_tensor("ones64", [1, DH], F32R, kind="ExternalInput").ap()
    if use_ln1b:
        lnb_d = nc.dram_tensor("lnb", [1, D], F32, kind="ExternalInput").ap()
    if use_qlw:
        qlw_d = nc.dram_tensor("qlw", [1, CD], F32, kind="ExternalInput").ap()
    if use_klw:
        klw_d = nc.dram_tensor("klw", [1, CD], F32, kind="ExternalInput").ap()
    out_d = nc.dram_tensor("out", [L, D], BF16, kind="ExternalOutput").ap()

    x_t_d = x_d.rearrange("(n p) d -> n p d", p=P)
    out_t_d = out_d.rearrange("(n p) d -> n p d", p=P)


    with tile.TileContext(nc) as tc, ExitStack() as ctx:
        cpool = ctx.enter_context(tc.tile_pool(name="cpool", bufs=1))
        small = ctx.enter_context(tc.tile_pool(name="small", bufs=4))
        pstp = ctx.enter_context(tc.tile_pool(name="pstp", bufs=1, space="PSUM"))
        dramp = ctx.enter_context(tc.tile_pool(name="dramp", bufs=1, space="DRAM"))

        # --- persistent SBUF ---
        # DMA issue time is serial per queue (~0.7us each): spread the initial
        # loads across engine queues so issue parallelizes and x tiles (on
        # sync) aren't stuck behind weight/table loads.
        identb = cpool.tile([P, P], BF16)
        nc.gpsimd.dma_start(identb, idb_d)
        eps_ap = cpool.tile([P, 1], F32)
        nc.vector.memset(eps_ap, EPS)
        ones64 = cpool.tile([1, DH], F32R)
        nc.gpsimd.dma_start(ones64, on64_d)

        wqk_sb = cpool.tile([P, KC, 512], BF16)
        wqk_f = wqk_sb.rearrange("p c n -> p (c n)")
        for i in range(4):
            sl = slice(i * KC * 128, (i + 1) * KC * 128)
            nc.scalar.dma_start(wqk_f[:, sl], wqk_d[:, sl])
        wv_sb = cpool.tile([P, KC, CD], BF16)
        wv_f = wv_sb.rearrange("p c n -> p (c n)")
        for i in range(2):
            sl = slice(i * KC * P, (i + 1) * KC * P)
            nc.scalar.dma_start(wv_f[:, sl], wv_d[:, sl])

        cos4_sb = cpool.tile([P, TT, CD], BF16)
        nc.gpsimd.dma_start(cos4_sb.rearrange("p t d -> p (t d)"), cos4_d)
        sl4_sb = cpool.tile([P, TT, CD // 2], BF16)
        nc.gpsimd.dma_start(sl4_sb.rearrange("p t d -> p (t d)"), sl4_d)
        sh4_sb = cpool.tile([P, TT, CD // 2], BF16)
        nc.gpsimd.dma_start(sh4_sb.rearrange("p t d -> p (t d)"), sh4_d)

        # qT/kT: rows 0:64 head dims (transposed), 64:69 mask rows; rows 69:127
        # never read (scores contract only 0:69) -> no zero fill needed.
        qT = cpool.tile([P, HPC, L], BF16)
        kT = cpool.tile([P, HPC, L], BF16)
        for hh in range(HPC):
            nc.gpsimd.dma_start(qT[DH:KROWS, hh, :], mq_d)
            nc.gpsimd.dma_start(kT[DH:KROWS, hh, :], mk_d)

        # v blocks: [128 keys, kt, h, 64 dims + ones col]
        v_sb = cpool.tile([P, TT * HPC * VB], BF16)
        v_blocks = v_sb.rearrange("p (t h d) -> p t h d", t=TT, h=HPC)
        nc.gpsimd.memset(v_blocks[:, :, :, DH : DH + 1], 1.0)

        r14_sb = cpool.tile([P, TT, CD], BF16)
        nc.gpsimd.dma_start(r14_sb.rearrange("p t d -> p (t d)"), r14_d)

        wo_sb = cpool.tile([P, CD // P, D], BF16)
        nc.gpsimd.dma_start(wo_sb.rearrange("p c n -> p (c n)"), wo_d)

        if use_ln1b:
            lnb_sb = cpool.tile([P, D], F32)
            nc.sync.dma_start(lnb_sb, lnb_d.partition_broadcast(P)[:, 0, :])
        if use_qlw:
            qlw_sb = cpool.tile([P, CD], F32)
            nc.sync.dma_start(qlw_sb, qlw_d.partition_broadcast(P)[:, 0, :])
        if use_klw:
            klw_sb = cpool.tile([P, CD], F32)
            nc.sync.dma_start(klw_sb, klw_d.partition_broadcast(P)[:, 0, :])

        q4_all = cpool.tile([P, TT, CD], BF16)
        k4_all = cpool.tile([P, TT, CD], BF16)
        rot_q = cpool.tile([P, TT, CD], BF16)
        rot_k = cpool.tile([P, TT, CD], BF16)
        stats_pack = cpool.tile([P, TT, 2, 2], F32)   # [s1, s2] per (t, q/k)
        allred = cpool.tile([P, TT, 2, 2], F32)
        junk = cpool.tile([P, CD], BF16)

        ibs = [dramp.tile([P, (hi - lo) * 4], F32, name=f"ib{i}")
               for i, (lo, hi) in enumerate(CH)]
        obs = [dramp.tile([P, (hi - lo) * 4], F32, name=f"ob{i}")
               for i, (lo, hi) in enumerate(CH)]

        p2 = ctx.enter_context(tc.tile_pool(name="p2", bufs=2))

        def rope_emit(src_ap, dst_ap, lo, nt, tag):
            """dst = src*cos + rothalf(src)*sin for `nt` token tiles at once
            (flat bf16 ops, batched to amortize per-op overhead; sin signed)."""
            qa = p2.tile([P, 4, CD], BF16, tag=f"qa{tag}", bufs=2)
            nc.vector.tensor_mul(qa[:, 0:nt, :], src_ap, cos4_sb[:, lo : lo + nt, :])
            qbt = p2.tile([P, 4, HPC, 2, DH // 2], BF16, tag=f"qb{tag}", bufs=2)
            srcv = src_ap.rearrange("p t (h s d) -> p t h s d", h=HPC, s=2)
            nc.gpsimd.tensor_mul(
                qbt[:, 0:nt, :, 0, :], srcv[:, :, :, 1, :],
                sl4_sb[:, lo : lo + nt, :].rearrange("p t (h d) -> p t h d", h=HPC),
            )
            nc.gpsimd.tensor_mul(
                qbt[:, 0:nt, :, 1, :], srcv[:, :, :, 0, :],
                sh4_sb[:, lo : lo + nt, :].rearrange("p t (h d) -> p t h d", h=HPC),
            )
            nc.vector.tensor_add(
                dst_ap, qa[:, 0:nt, :],
                qbt[:, 0:nt, :, :, :].rearrange("p t h s d -> p t (h s d)"),
            )

        def finalize_stats(lo, hi):
            """qk-LN mu/rstd from the all-reduced sums for tiles [lo, hi)."""
            n = hi - lo
            mu = small.tile([P, n, 2], F32, tag="fmu", name=f"fmu{lo}")
            nc.vector.tensor_scalar(mu, allred[:, lo:hi, :, 0], 1.0 / D, None, ALU.mult)
            m2 = small.tile([P, n, 2], F32, tag="fm2", name=f"fm2{lo}")
            nc.vector.tensor_mul(m2, mu, mu)
            var = small.tile([P, n, 2], F32, tag="fvar", name=f"fvar{lo}")
            nc.vector.scalar_tensor_tensor(
                var, allred[:, lo:hi, :, 1], 1.0 / D, m2,
                op0=ALU.mult, op1=ALU.subtract,
            )
            rstd = small.tile([P, n, 2], F32, tag="frstd", name=f"frstd{lo}")
            nc.scalar.activation(rstd, var, AF.Sqrt, bias=eps_ap)
            nc.vector.reciprocal(rstd, rstd)
            nm = small.tile([P, n, 2], F32, tag="fnm", name=f"fnm{lo}")
            nc.vector.scalar_tensor_tensor(nm, mu, -1.0, rstd, op0=ALU.mult, op1=ALU.mult)
            return rstd, nm

        def finalize_fold(t, j, rstd, nm, lo):
            """Fold LN affine into rope'd q/k for tile t, build qT/kT columns."""
            rs = rstd[:, t - lo, j : j + 1]
            nmj = nm[:, t - lo, j : j + 1]
            lw_flag = use_qlw if j == 0 else use_klw
            foldt = p2.tile([P, CD], BF16, tag="fold", bufs=3)
            if lw_flag:
                src4 = q4_all if j == 0 else k4_all
                lw_sb = qlw_sb if j == 0 else klw_sb
                xn = p2.tile([P, 1, CD], BF16, tag="xn", bufs=2)
                nc.scalar.activation(
                    xn[:, 0, :], src4[:, t, :], AF.Identity, bias=nmj, scale=rs
                )
                nc.vector.tensor_mul(xn[:, 0, :], xn[:, 0, :], lw_sb)
                rope_emit(xn, foldt.rearrange("p (t d) -> p t d", t=1), t, 1, "f")
            else:
                rot = rot_q if j == 0 else rot_k
                tmp = p2.tile([P, CD], BF16, tag="ftmp", bufs=2)
                nmb = bass.AP(tensor=nmj.tensor, offset=nmj.offset,
                              ap=[nmj.ap[0], [0, CD]])
                nc.gpsimd.tensor_mul(tmp, r14_sb[:, t, :], nmb)
                nc.vector.scalar_tensor_tensor(
                    foldt, rot[:, t, :], rs, tmp, op0=ALU.mult, op1=ALU.add
                )
            tp = pstp.tile([DH, HPC, P], BF16, tag="tp", bufs=2)
            for hh in range(HPC):
                nc.tensor.transpose(
                    tp[:, hh, :], foldt[:, hh * DH : (hh + 1) * DH], identb
                )
            dst = qT if j == 0 else kT
            nc.vector.tensor_copy(dst[0:DH, :, t * P : (t + 1) * P], tp)

        finalizeA_state = []

        # ================= Phase 1: LN1 + QKV + raw rope + stats =================
        with ExitStack() as phA:
            p1 = phA.enter_context(tc.tile_pool(name="p1", bufs=2))
            psA = phA.enter_context(tc.tile_pool(name="psA", bufs=1, space="PSUM"))

            xtiles = {}

            def xfetch(t):
                if t >= TT or t in xtiles:
                    return
                xt = p1.tile([P, D], F32, tag="x", bufs=4, name=f"x{t}")
                nc.sync.dma_start(xt[:, 0:512], x_t_d[t][:, 0:512])
                nc.sync.dma_start(xt[:, 512:1024], x_t_d[t][:, 512:1024])
                xtiles[t] = xt

            for t in range(3):
                xfetch(t)

            for t in range(TT):
                xfetch(t + 3)
                x_t = xtiles.pop(t)

                # LN1 stats
                xstats = small.tile([P, 2, 6], F32, tag="xstats")
                for s in range(2):
                    nc.vector.bn_stats(
                        xstats[:, s, :],
                        x_t[:, s * 512 : (s + 1) * 512].rearrange(
                            "p (s d) -> p s d", s=1
                        ),
                    )
                xmv = small.tile([P, 2], F32, tag="xmv")
                nc.vector.bn_aggr(xmv, xstats)
                xrstd = small.tile([P, 1], F32, tag="xrstd")
                nc.scalar.activation(xrstd, xmv[:, 1:2], AF.Sqrt, bias=eps_ap)
                nc.vector.reciprocal(xrstd, xrstd)
                xnm = small.tile([P, 1], F32, tag="xnm")
                nc.vector.tensor_scalar(xnm, xmv[:, 0:1], xrstd, -1.0, ALU.mult, ALU.mult)
                h_t = p1.tile([P, D], BF16, tag="h", bufs=2)
                nc.scalar.activation(h_t, x_t, AF.Identity, bias=xnm, scale=xrstd)
                if use_ln1b:
                    nc.vector.tensor_add(h_t, h_t, lnb_sb)

                # hT (bf16 transposes)
                ht_ps = psA.tile([P, KC, P], BF16, tag="ht", bufs=2)
                for c in range(KC):
                    nc.tensor.transpose(ht_ps[:, c, :], h_t[:, c * P : (c + 1) * P], identb)
                hT_t = p1.tile([P, KC, P], BF16, tag="hT", bufs=2)
                nc.scalar.copy(hT_t, ht_ps)

                # QKV
                qk_ps = psA.tile([P, 512], F32, tag="qk", bufs=2)
                v_ps = psA.tile([P, CD], F32, tag="v", bufs=2)
                for c in range(KC):
                    nc.tensor.matmul(qk_ps, hT_t[:, c, :], wqk_sb[:, c, :],
                                     start=(c == 0), stop=(c == KC - 1))
                    nc.tensor.matmul(v_ps, hT_t[:, c, :], wv_sb[:, c, :],
                                     start=(c == 0), stop=(c == KC - 1))

                # psum -> sbuf copies; s1/s2 accumulate for free on ACT
                nc.scalar.activation(
                    q4_all[:, t, :], qk_ps[:, 0:CD], AF.Copy,
                    accum_out=stats_pack[:, t, 0, 0:1],
                )
                nc.scalar.activation(
                    k4_all[:, t, :], qk_ps[:, CD:512], AF.Copy,
                    accum_out=stats_pack[:, t, 1, 0:1],
                )
                nc.vector.scalar_tensor_tensor(
                    junk, q4_all[:, t, :], 1.0, q4_all[:, t, :],
                    op0=ALU.mult, op1=ALU.mult,
                    accum_out=stats_pack[:, t, 0, 1:2],
                )
                nc.vector.scalar_tensor_tensor(
                    junk, k4_all[:, t, :], 1.0, k4_all[:, t, :],
                    op0=ALU.mult, op1=ALU.mult,
                    accum_out=stats_pack[:, t, 1, 1:2],
                )
                nc.scalar.activation(
                    v_blocks[:, t, :, 0:DH],
                    v_ps.rearrange("p (h d) -> p h d", h=HPC), AF.Copy,
                )

                # raw rope (LN affine folded in later), batched per 4 tiles
                if t % 4 == 3:
                    g = t - 3
                    if not use_qlw:
                        rope_emit(q4_all[:, g : t + 1, :], rot_q[:, g : t + 1, :],
                                  g, 4, "q")
                    if not use_klw:
                        rope_emit(k4_all[:, g : t + 1, :], rot_k[:, g : t + 1, :],
                                  g, 4, "k")

                for ci, (lo, hi) in enumerate(CH):
                    if t == hi - 1:
                        nc.gpsimd.dma_start(
                            ibs[ci],
                            stats_pack[:, lo:hi, :, :].rearrange("p t j s -> p (t j s)"),
                        )
                        nc.gpsimd.collective_compute(
                            "AllReduce", ALU.add, replica_groups=RG,
                            ins=[ibs[ci].opt()], outs=[obs[ci].opt()],
                        )
                        nc.sync.dma_start(
                            allred[:, lo:hi, :, :].rearrange("p t j s -> p (t j s)"),
                            obs[ci],
                        )
                if t == 12 and not (use_qlw or use_klw):
                    # hide the finalize of the first stats chunk under the last
                    # phase-1 tiles (fold math runs on the idle GpSimd engine)
                    finalizeA_state.append(finalize_stats(*CH[0]))
                    for ft in range(*CH[0]):
                        for j in range(2):
                            finalize_fold(ft, j, *finalizeA_state[0], CH[0][0])

        # ================= Phase 2: sparse attention =================
        with ExitStack() as phB:
            ps2 = phB.enter_context(tc.tile_pool(name="ps2", bufs=1, space="PSUM"))

            def attn_qb(qb):
                kts = kts_per_qb[qb]
                pairs = [kts[i : i + 2] for i in range(0, len(kts), 2)]
                npair = len(pairs)
                ctxT = p2.tile([P, 2, QB], BF16, tag="ctxT", bufs=2, name=f"ctxT{qb}")
                for hp in range(2):
                    ctx = ps2.tile([VB, 2, QB], F32, tag=f"ctx{hp}", bufs=1,
                                   name=f"ctx{qb}_{hp}")
                    for jj in range(2):
                        h = 2 * hp + jj
                        pend = None

                        def emit_pv(pi, pair, eT):
                            for i, kt in enumerate(pair):
                                nc.tensor.matmul(
                                    ctx[:, jj, :],
                                    v_sb[:, (kt * HPC + h) * VB : (kt * HPC + h) * VB + VB],
                                    eT[:, i, :],
                                    start=(pi == 0 and i == 0),
                                    stop=(pi == npair - 1 and i == len(pair) - 1),
                                )

                        for pi, pair in enumerate(pairs):
                            w = len(pair)
                            s_ps = ps2.tile([P, 2, QB], F32, tag="sc", bufs=3)
                            for i, kt in enumerate(pair):
                                nc.tensor.matmul(
                                    s_ps[:, i, :],
                                    kT[0:KROWS, h, kt * P : (kt + 1) * P],
                                    qT[0:KROWS, h, qb * QB : (qb + 1) * QB],
                                    start=True, stop=True,
                                )
                            eT = p2.tile([P, 2, QB], BF16, tag="eT", bufs=3)
                            nc.scalar.activation(eT[:, 0:w, :], s_ps[:, 0:w, :], AF.Exp)
                            if pend is not None:
                                emit_pv(*pend)
                            pend = (pi, pair, eT)
                        emit_pv(*pend)

                    # normalize: broadcast the denominator row to 64 rows by
                    # DMA (psum -> sbuf, stride-0 partition), then fast approx
                    # reciprocal (~18 bits, plenty for softmax)
                    z_sb = small.tile([1, 2, QB], F32, tag="z", name=f"z{qb}_{hp}")
                    nc.vector.tensor_copy(z_sb, ctx[DH : DH + 1, :, :])
                    zr_f = small.tile([1, 2, QB], F32, tag="zrf", name=f"zrf{qb}_{hp}")
                    nc.vector.reciprocal_approx_fast(zr_f, z_sb)
                    zr = small.tile([1, 2, QB], F32R, tag="zr", name=f"zr{qb}_{hp}")
                    nc.vector.tensor_copy(zr, zr_f)
                    repl = ps2.tile([DH, 2, QB], F32, tag="repl", bufs=1,
                                    name=f"repl{qb}_{hp}")
                    nc.tensor.matmul(
                        repl.rearrange("p a b -> p (a b)"), ones64,
                        zr.rearrange("p a b -> p (a b)"),
                        start=True, stop=True,
                    )
                    repl_sb = p2.tile([DH, 2, QB], F32, tag="replsb", bufs=2,
                                      name=f"replsb{qb}_{hp}")
                    nc.scalar.copy(repl_sb, repl)
                    for jj in range(2):
                        nc.vector.tensor_mul(
                            ctxT[jj * DH : (jj + 1) * DH, hp, :],
                            ctx[0:DH, jj, :],
                            repl_sb[:, jj, :],
                        )
                return ctxT

            def outproj_qb(qb, ctxT):
                # out projection for a q block (256 tokens = 2 out tiles)
                for tt in range(2):
                    gt = qb * 2 + tt
                    o_sb = p2.tile([P, D], BF16, tag="osb", bufs=2, name=f"osb{gt}")
                    for s in range(2):
                        o_ps = ps2.tile([P, 2, QB], F32, tag="sc", bufs=3,
                                        name=f"o{gt}_{s}")
                        ov = o_ps.rearrange("p a b -> p (a b)")
                        for c in range(2):
                            nc.tensor.matmul(
                                ov,
                                ctxT[:, c, tt * P : (tt + 1) * P],
                                wo_sb[:, c, s * 512 : (s + 1) * 512],
                                start=(c == 0), stop=(c == 1),
                            )
                        if s == 0:
                            nc.scalar.copy(o_sb[:, 0:512], ov)
                        else:
                            nc.vector.tensor_copy(o_sb[:, 512:1024], ov)
                    nc.sync.dma_start(out_t_d[gt], o_sb)

            # qbs grouped by the stats chunk covering their largest k-tile;
            # software-pipeline the out-projection one q-block behind attention
            # so PE never idles on the (DVE) normalize chain
            done = set()
            pending = None
            for ci, (lo, hi) in enumerate(CH):
                if ci > 0 or not finalizeA_state:
                    st = finalize_stats(lo, hi)
                    for ft in range(lo, hi):
                        for j in range(2):
                            finalize_fold(ft, j, *st, lo)
                for qb in range(NQB):
                    if qb in done or kts_per_qb[qb][-1] >= hi:
                        continue
                    done.add(qb)
                    ctxT = attn_qb(qb)
                    if pending is not None:
                        outproj_qb(*pending)
                    pending = (qb, ctxT)
            if pending is not None:
                outproj_qb(*pending)

    nc.compile()
    return nc


_CACHE = {}


def _get_nc(key):
    if key not in _CACHE:
        _CACHE[key] = build_bass(*key)
    return _CACHE[key]


def _plan(seq_id):
    """Compile-time sparsity plan from seq_id (union over both batches)."""
    kts_per_qb = []
    for qb in range(NQB):
        s = set()
        for b in range(B):
            sid = seq_id[b]
            segs = set(int(v) for v in sid[qb * QB : (qb + 1) * QB])
            for kt in range(TT):
                ksegs = set(int(v) for v in sid[kt * P : (kt + 1) * P])
                if ksegs & segs:
                    s.add(kt)
        kts_per_qb.append(tuple(sorted(s)))
    return (tuple(kts_per_qb),)


def _host_prep(x, seq_id, ln1_w, ln1_b, w_qkv, q_ln_w, k_ln_w, w_out):
    """Build the 8 per-core input maps + compile key."""
    x = np.asarray(x, np.float32)
    seq_id = np.asarray(seq_id)
    ln1_w = np.asarray(ln1_w, np.float32)
    ln1_b = np.asarray(ln1_b, np.float32)
    w_qkv = np.asarray(w_qkv, np.float32)
    q_ln_w = np.asarray(q_ln_w, np.float32)
    k_ln_w = np.asarray(k_ln_w, np.float32)
    w_out = np.asarray(w_out, np.float32)

    use_ln1b = bool(np.any(ln1_b != 0.0))
    use_qlw = not np.allclose(q_ln_w, 1.0)
    use_klw = not np.allclose(k_ln_w, 1.0)

    bf = ml_dtypes.bfloat16
    wq_f = w_qkv[:, 0:D] * ln1_w[:, None]
    wk_f = w_qkv[:, D : 2 * D] * ln1_w[:, None]
    wv_f = w_qkv[:, 2 * D : 3 * D] * ln1_w[:, None]

    # rope tables with 1/sqrt(sqrt(64)) on each side -> scores * 1/8;
    # replicated over the 4 local heads for flat 2D device ops
    inv_freq = 1.0 / (ROPE_BASE ** (np.arange(0, DH, 2, dtype=np.float32) / DH))
    tpos = np.arange(L, dtype=np.float32)
    freqs = np.einsum("l,f->lf", tpos, inv_freq)
    emb = np.concatenate([freqs, freqs], axis=-1)
    s8 = np.float32(8.0 ** -0.5)
    cos_t = (np.cos(emb) * s8).astype(np.float32)
    sin_t = (np.sin(emb) * s8).astype(np.float32)
    r1 = np.concatenate(
        [cos_t[:, : DH // 2] - sin_t[:, : DH // 2],
         cos_t[:, DH // 2 :] + sin_t[:, DH // 2 :]], axis=1
    )
    def pack(a):
        """[n*128, W] -> [128, n*W] partition-major (device tile [p, n, W])."""
        n = a.shape[0] // P
        return np.ascontiguousarray(
            a.reshape(n, P, a.shape[1]).transpose(1, 0, 2).reshape(P, -1)
        )

    cos4 = pack(np.tile(cos_t, (1, HPC))).astype(bf)
    sl4 = pack(np.tile(-sin_t[:, : DH // 2], (1, HPC))).astype(bf)
    sh4 = pack(np.tile(sin_t[:, DH // 2 :], (1, HPC))).astype(bf)
    r14 = pack(np.tile(r1, (1, HPC))).astype(bf)

    identb = np.eye(P, dtype=bf)
    ones64 = np.ones((1, DH), np.float32)

    (kts_per_qb,) = _plan(seq_id)
    key = (use_ln1b, use_qlw, use_klw, kts_per_qb)

    in_maps = []
    for c in range(8):
        b, g = c // HPC, c % HPC
        mine = np.arange(g * CD, (g + 1) * CD)

        sid = np.asarray(seq_id[b], np.int64)
        A = (sid[None, :] == np.arange(4)[:, None]).astype(np.float32)
        maskq = np.concatenate([MASK_A * A, MASK_A * np.ones((1, L), np.float32)])
        maskk = np.concatenate([MASK_A * A, -MASK_A * np.ones((1, L), np.float32)])

        m = {
            "x": np.ascontiguousarray(x[b]),
            "wqk": pack(
                np.concatenate([wq_f[:, mine], wk_f[:, mine]], axis=1)
            ).astype(bf),
            "wv": pack(wv_f[:, mine]).astype(bf),
            "wo": pack(w_out[mine, :]).astype(bf),
            "maskq": maskq.astype(bf),
            "maskk": maskk.astype(bf),
            "cos4": cos4,
            "sl4": sl4,
            "sh4": sh4,
            "r14": r14,
            "identb": identb,
            "ones64": ones64,
        }
        if use_ln1b:
            m["lnb"] = ln1_b.reshape(1, D)
        if use_qlw:
            m["qlw"] = q_ln_w[mine].reshape(1, CD)
        if use_klw:
            m["klw"] = k_ln_w[mine].reshape(1, CD)
        in_maps.append(m)
    return in_maps, key


def run(inputs, trace=False):
    """Run on hardware; returns (output [B, L, D] fp32, BassKernelResults)."""
    in_maps, key = _host_prep(**inputs)
    nc = _get_nc(key)
    res = bass_utils.run_bass_kernel_spmd(
        nc, in_maps, core_ids=list(range(8)), trace=trace
    )
    out = np.zeros((B, L, D), np.float32)
    for c in range(8):
        out[c // HPC] += np.asarray(res.results[c]["out"], dtype=np.float32)
    return out, res


def kernel(**inputs) -> np.ndarray:
    out, _ = run(inputs)
    return out


# revision 29
# speedup vs baseline: 1.0943x; 1.0943x over previous
"""Trainium2 Bass kernel for MHA block (LN -> QKV -> qk-LN -> RoPE -> masked attn -> out-proj).

Self-contained: hardcodes shapes B=2, L=2048, D=1024, H=16, Dh=64; runs on 8 NeuronCores
via bass_utils.run_bass_kernel_spmd. Sharding: core c = (batch b = c//4, head-group
g = c%4 of 4 heads). Weight columns are sliced per core so "our" 4 heads are always
columns 0:256 -> the device program is identical on all cores (SPMD).

Key structure (v2):
- bf16 compute throughout (weights, h, q/k/v, probs, ctx, out partials); fp32 PSUM.
- Sparse attention: seq_id is sorted per batch -> the mask is block diagonal. The
  kernel is compiled per seq-segment structure (computed from the actual input in
  kernel()); (q-block, k-tile) pairs with no segment overlap in EITHER batch are
  skipped entirely (scores, exp and PV). Masks rows (5 extra contraction rows at
  64:69 of qT/kT) give exact masking at segment boundaries.
- qk-LN stats (over the full 1024 dims) come from per-core partial sums gathered
  free on the ACT copy/square accumulators, with two 4-core-group AllReduces.
  Attention q-blocks are ordered so blocks only needing k-tiles < SP run while the
  second AllReduce is in flight.
- rstd = exp(-0.5*ln(var+eps)): Ln/Exp/Identity/Copy/Square all live in ONE ACT
  table -> no table-swap stalls between LN work and softmax exp.
- RoPE applied to raw q/k (linear), LN affine folded in afterwards:
  rot(LN(q)) = rstd*rot(q) + (-rstd*mu)*rot(ones); rope/fold are flat 2D bf16 ops
  with host-replicated tables.
"""

import numpy as np
import ml_dtypes
from contextlib import ExitStack

import concourse.bass as bass
import concourse.tile as tile
from concourse import bacc, mybir
from concourse import bass_utils

F32 = mybir.dt.float32
F32R = mybir.dt.float32r
BF16 = mybir.dt.bfloat16
AF = mybir.ActivationFunctionType
ALU = mybir.AluOpType

B, L, D = 2, 2048, 1024
H, DH = 16, 64
HPC = 4          # heads per core
CD = HPC * DH    # ctx dims per core = 256
P = 128
TT = L // P      # 16 token tiles
KC = D // P      # 8 contraction chunks
QB = 256         # query block width
NQB = L // QB    # 8 query blocks
EPS = 1e-5
ROPE_BASE = 10000.0
MASK_A = 8.0     # mask row scale; mask bias = -MASK_A^2 = -64 for masked pairs
KROWS = DH + 5   # contraction rows for scores: 64 dims + 5 mask rows
VB = DH + 1      # v block width (64 dims + ones col)
RG = [[0, 1, 2, 3], [4, 5, 6, 7]]


def build_bass(use_ln1b, use_qlw, use_klw, kts_per_qb):
    # stats-AllReduce chunks; a tiny barrier collective at kernel start
    # absorbs the inter-core launch skew so these land fast
    CH = [(0, 9), (9, 16)]
    nc = bacc.Bacc("TRN2", target_bir_lowering=False, debug=False, num_devices=8)

    # ---- DRAM I/O ----
    # weights/tables arrive host-prepacked partition-major ([128, contiguous])
    # so every DMA is 128 simple full lines: fast descriptor issue + bandwidth
    x_d = nc.dram_tensor("x", [L, D], F32, kind="ExternalInput").ap()
    wqk_d = nc.dram_tensor("wqk", [P, KC * 512], BF16, kind="ExternalInput").ap()
    wv_d = nc.dram_tensor("wv", [P, KC * CD], BF16, kind="ExternalInput").ap()
    wo_d = nc.dram_tensor("wo", [P, 2 * D], BF16, kind="ExternalInput").ap()
    mq_d = nc.dram_tensor("maskq", [5, L], BF16, kind="ExternalInput").ap()
    mk_d = nc.dram_tensor("maskk", [5, L], BF16, kind="ExternalInput").ap()
    cos4_d = nc.dram_tensor("cos4", [P, TT * CD], BF16, kind="ExternalInput").ap()
    sl4_d = nc.dram_tensor("sl4", [P, TT * CD // 2], BF16, kind="ExternalInput").ap()
    sh4_d = nc.dram_tensor("sh4", [P, TT * CD // 2], BF16, kind="ExternalInput").ap()
    r14_d = nc.dram_tensor("r14", [P, TT * CD], BF16, kind="ExternalInput").ap()
    idb_d = nc.dram_tensor("identb", [P, P], BF16, kind="ExternalInput").ap()
    on64_d = nc.dram_tensor("ones64", [1, DH], F32R, kind="ExternalInput").ap()
    if use_ln1b:
        lnb_d = nc.dram_tensor("lnb", [1, D], F32, kind="ExternalInput").ap()
    if use_qlw:
        qlw_d = nc.dram_tensor("qlw", [1, CD], F32, kind="ExternalInput").ap()
    if use_klw:
        klw_d = nc.dram_tensor("klw", [1, CD], F32, kind="ExternalInput").ap()
    out_d = nc.dram_tensor("out", [L, D], BF16, kind="ExternalOutput").ap()

    x_t_d = x_d.rearrange("(n p) d -> n p d", p=P)
    out_t_d = out_d.rearrange("(n p) d -> n p d", p=P)


    with tile.TileContext(nc) as tc, ExitStack() as ctx:
        cpool = ctx.enter_context(tc.tile_pool(name="cpool", bufs=1))
        small = ctx.enter_context(tc.tile_pool(name="small", bufs=4))
        pstp = ctx.enter_context(tc.tile_pool(name="pstp", bufs=1, space="PSUM"))
        dramp = ctx.enter_context(tc.tile_pool(name="dramp", bufs=1, space="DRAM"))

        # --- persistent SBUF ---
        # DMA issue time is serial per queue (~0.7us each): spread the initial
        # loads across engine queues so issue parallelizes and x tiles (on
        # sync) aren't stuck behind weight/table loads.
        identb = cpool.tile([P, P], BF16)
        nc.gpsimd.dma_start(identb, idb_d)
        eps_ap = cpool.tile([P, 1], F32)
        nc.vector.memset(eps_ap, EPS)
        ones64 = cpool.tile([1, DH], F32R)
        nc.gpsimd.dma_start(ones64, on64_d)

        wqk_sb = cpool.tile([P, KC, 512], BF16)
        wqk_f = wqk_sb.rearrange("p c n -> p (c n)")
        for i in range(4):
            sl = slice(i * KC * 128, (i + 1) * KC * 128)
            nc.scalar.dma_start(wqk_f[:, sl], wqk_d[:, sl])
        wv_sb = cpool.tile([P, KC, CD], BF16)
        wv_f = wv_sb.rearrange("p c n -> p (c n)")
        for i in range(2):
            sl = slice(i * KC * P, (i + 1) * KC * P)
            nc.scalar.dma_start(wv_f[:, sl], wv_d[:, sl])

        cos4_sb = cpool.tile([P, TT, CD], BF16)
        nc.gpsimd.dma_start(cos4_sb.rearrange("p t d -> p (t d)"), cos4_d)
        sl4_sb = cpool.tile([P, TT, CD // 2], BF16)
        nc.gpsimd.dma_start(sl4_sb.rearrange("p t d -> p (t d)"), sl4_d)
        sh4_sb = cpool.tile([P, TT, CD // 2], BF16)
        nc.gpsimd.dma_start(sh4_sb.rearrange("p t d -> p (t d)"), sh4_d)

        # qT/kT: rows 0:64 head dims (transposed), 64:69 mask rows; rows 69:127
        # never read (scores contract only 0:69) -> no zero fill needed.
        qT = cpool.tile([P, HPC, L], BF16)
        kT = cpool.tile([P, HPC, L], BF16)
        for hh in range(HPC):
            nc.gpsimd.dma_start(qT[DH:KROWS, hh, :], mq_d)
            nc.gpsimd.dma_start(kT[DH:KROWS, hh, :], mk_d)

        # v blocks: [128 keys, kt, h, 64 dims + ones col]
        v_sb = cpool.tile([P, TT * HPC * VB], BF16)
        v_blocks = v_sb.rearrange("p (t h d) -> p t h d", t=TT, h=HPC)
        nc.gpsimd.memset(v_blocks[:, :, :, DH : DH + 1], 1.0)

        r14_sb = cpool.tile([P, TT, CD], BF16)
        nc.gpsimd.dma_start(r14_sb.rearrange("p t d -> p (t d)"), r14_d)

        wo_sb = cpool.tile([P, CD // P, D], BF16)
        nc.gpsimd.dma_start(wo_sb.rearrange("p c n -> p (c n)"), wo_d)

        if use_ln1b:
            lnb_sb = cpool.tile([P, D], F32)
            nc.sync.dma_start(lnb_sb, lnb_d.partition_broadcast(P)[:, 0, :])
        if use_qlw:
            qlw_sb = cpool.tile([P, CD], F32)
            nc.sync.dma_start(qlw_sb, qlw_d.partition_broadcast(P)[:, 0, :])
        if use_klw:
            klw_sb = cpool.tile([P, CD], F32)
            nc.sync.dma_start(klw_sb, klw_d.partition_broadcast(P)[:, 0, :])

        q4_all = cpool.tile([P, TT, CD], BF16)
        k4_all = cpool.tile([P, TT, CD], BF16)
        rot_q = cpool.tile([P, TT, CD], BF16)
        rot_k = cpool.tile([P, TT, CD], BF16)
        stats_pack = cpool.tile([P, TT, 2, 2], F32)   # [s1, s2] per (t, q/k)
        allred = cpool.tile([P, TT, 2, 2], F32)
        junk = cpool.tile([P, CD], BF16)

        ibs = [dramp.tile([P, (hi - lo) * 4], F32, name=f"ib{i}")
               for i, (lo, hi) in enumerate(CH)]
        obs = [dramp.tile([P, (hi - lo) * 4], F32, name=f"ob{i}")
               for i, (lo, hi) in enumerate(CH)]

        p2 = ctx.enter_context(tc.tile_pool(name="p2", bufs=2))

        def rope_emit(src_ap, dst_ap, lo, nt, tag):
            """dst = src*cos + rothalf(src)*sin for `nt` token tiles at once
            (flat bf16 ops, batched to amortize per-op overhead; sin signed)."""
            qa = p2.tile([P, 4, CD], BF16, tag=f"qa{tag}", bufs=2)
            nc.vector.tensor_mul(qa[:, 0:nt, :], src_ap, cos4_sb[:, lo : lo + nt, :])
            qbt = p2.tile([P, 4, HPC, 2, DH // 2], BF16, tag=f"qb{tag}", bufs=2)
            srcv = src_ap.rearrange("p t (h s d) -> p t h s d", h=HPC, s=2)
            nc.gpsimd.tensor_mul(
                qbt[:, 0:nt, :, 0, :], srcv[:, :, :, 1, :],
                sl4_sb[:, lo : lo + nt, :].rearrange("p t (h d) -> p t h d", h=HPC),
            )
            nc.gpsimd.tensor_mul(
                qbt[:, 0:nt, :, 1, :], srcv[:, :, :, 0, :],
                sh4_sb[:, lo : lo + nt, :].rearrange("p t (h d) -> p t h d", h=HPC),
            )
            nc.vector.tensor_add(
                dst_ap, qa[:, 0:nt, :],
                qbt[:, 0:nt, :, :, :].rearrange("p t h s d -> p t (h s d)"),
            )

        def finalize_stats(lo, hi):
            """qk-LN mu/rstd from the all-reduced sums for tiles [lo, hi)."""
            n = hi - lo
            mu = small.tile([P, n, 2], F32, tag="fmu", name=f"fmu{lo}")
            nc.vector.tensor_scalar(mu, allred[:, lo:hi, :, 0], 1.0 / D, None, ALU.mult)
            m2 = small.tile([P, n, 2], F32, tag="fm2", name=f"fm2{lo}")
            nc.vector.tensor_mul(m2, mu, mu)
            var = small.tile([P, n, 2], F32, tag="fvar", name=f"fvar{lo}")
            nc.vector.scalar_tensor_tensor(
                var, allred[:, lo:hi, :, 1], 1.0 / D, m2,
                op0=ALU.mult, op1=ALU.subtract,
            )
            rstd = small.tile([P, n, 2], F32, tag="frstd", name=f"frstd{lo}")
            nc.scalar.activation(rstd, var, AF.Sqrt, bias=eps_ap)
            nc.vector.reciprocal(rstd, rstd)
            nm = small.tile([P, n, 2], F32, tag="fnm", name=f"fnm{lo}")
            nc.vector.scalar_tensor_tensor(nm, mu, -1.0, rstd, op0=ALU.mult, op1=ALU.mult)
            return rstd, nm

        def finalize_fold(t, j, rstd, nm, lo):
            """Fold LN affine into rope'd q/k for tile t, build qT/kT columns."""
            rs = rstd[:, t - lo, j : j + 1]
            nmj = nm[:, t - lo, j : j + 1]
            lw_flag = use_qlw if j == 0 else use_klw
            foldt = p2.tile([P, CD], BF16, tag="fold", bufs=3)
            if lw_flag:
                src4 = q4_all if j == 0 else k4_all
                lw_sb = qlw_sb if j == 0 else klw_sb
                xn = p2.tile([P, 1, CD], BF16, tag="xn", bufs=2)
                nc.scalar.activation(
                    xn[:, 0, :], src4[:, t, :], AF.Identity, bias=nmj, scale=rs
                )
                nc.vector.tensor_mul(xn[:, 0, :], xn[:, 0, :], lw_sb)
                rope_emit(xn, foldt.rearrange("p (t d) -> p t d", t=1), t, 1, "f")
            else:
                rot = rot_q if j == 0 else rot_k
                tmp = p2.tile([P, CD], BF16, tag="ftmp", bufs=2)
                nmb = bass.AP(tensor=nmj.tensor, offset=nmj.offset,
                              ap=[nmj.ap[0], [0, CD]])
                nc.gpsimd.tensor_mul(tmp, r14_sb[:, t, :], nmb)
                nc.vector.scalar_tensor_tensor(
                    foldt, rot[:, t, :], rs, tmp, op0=ALU.mult, op1=ALU.add
                )
            tp = pstp.tile([DH, HPC, P], BF16, tag="tp", bufs=2)
            for hh in range(HPC):
                nc.tensor.transpose(
                    tp[:, hh, :], foldt[:, hh * DH : (hh + 1) * DH], identb
                )
            dst = qT if j == 0 else kT
            nc.vector.tensor_copy(dst[0:DH, :, t * P : (t + 1) * P], tp)

        finalizeA_state = []

        # barrier: 512B AllReduce at t~0 absorbs inter-core launch skew while
        # every engine is idle waiting on the initial DMAs; later stats
        # AllReduces then only wait on genuine compute-time differences
        bar_sb = cpool.tile([P, 1], F32)
        nc.vector.memset(bar_sb, 1.0)
        ibB = dramp.tile([P, 1], F32, name="ibB")
        obB = dramp.tile([P, 1], F32, name="obB")
        nc.gpsimd.dma_start(ibB, bar_sb)
        nc.gpsimd.collective_compute(
            "AllReduce", ALU.add, replica_groups=RG,
            ins=[ibB.opt()], outs=[obB.opt()],
        )
        nc.sync.dma_start(bar_sb, obB)

        # ================= Phase 1: LN1 + QKV + raw rope + stats =================
        with ExitStack() as phA:
            p1 = phA.enter_context(tc.tile_pool(name="p1", bufs=2))
            psA = phA.enter_context(tc.tile_pool(name="psA", bufs=1, space="PSUM"))

            xtiles = {}

            def xfetch(t):
                if t >= TT or t in xtiles:
                    return
                xt = p1.tile([P, D], F32, tag="x", bufs=4, name=f"x{t}")
                nc.sync.dma_start(xt[:, 0:512], x_t_d[t][:, 0:512])
                nc.sync.dma_start(xt[:, 512:1024], x_t_d[t][:, 512:1024])
                xtiles[t] = xt

            for t in range(3):
                xfetch(t)

            for t in range(TT):
                xfetch(t + 3)
                x_t = xtiles.pop(t)

                # LN1 stats
                xstats = small.tile([P, 2, 6], F32, tag="xstats")
                for s in range(2):
                    nc.vector.bn_stats(
                        xstats[:, s, :],
                        x_t[:, s * 512 : (s + 1) * 512].rearrange(
                            "p (s d) -> p s d", s=1
                        ),
                    )
                xmv = small.tile([P, 2], F32, tag="xmv")
                nc.vector.bn_aggr(xmv, xstats)
                xrstd = small.tile([P, 1], F32, tag="xrstd")
                nc.scalar.activation(xrstd, xmv[:, 1:2], AF.Sqrt, bias=eps_ap)
                nc.vector.reciprocal(xrstd, xrstd)
                xnm = small.tile([P, 1], F32, tag="xnm")
                nc.vector.tensor_scalar(xnm, xmv[:, 0:1], xrstd, -1.0, ALU.mult, ALU.mult)
                h_t = p1.tile([P, D], BF16, tag="h", bufs=2)
                nc.scalar.activation(h_t, x_t, AF.Identity, bias=xnm, scale=xrstd)
                if use_ln1b:
                    nc.vector.tensor_add(h_t, h_t, lnb_sb)

                # hT (bf16 transposes)
                ht_ps = psA.tile([P, KC, P], BF16, tag="ht", bufs=2)
                for c in range(KC):
                    nc.tensor.transpose(ht_ps[:, c, :], h_t[:, c * P : (c + 1) * P], identb)
                hT_t = p1.tile([P, KC, P], BF16, tag="hT", bufs=2)
                nc.vector.tensor_copy(hT_t, ht_ps)

                # QKV
                qk_ps = psA.tile([P, 512], F32, tag="qk", bufs=2)
                v_ps = psA.tile([P, CD], F32, tag="v", bufs=2)
                for c in range(KC):
                    nc.tensor.matmul(qk_ps, hT_t[:, c, :], wqk_sb[:, c, :],
                                     start=(c == 0), stop=(c == KC - 1))
                    nc.tensor.matmul(v_ps, hT_t[:, c, :], wv_sb[:, c, :],
                                     start=(c == 0), stop=(c == KC - 1))

                # psum -> sbuf copies; s1/s2 accumulate for free on ACT
                nc.scalar.activation(
                    q4_all[:, t, :], qk_ps[:, 0:CD], AF.Copy,
                    accum_out=stats_pack[:, t, 0, 0:1],
                )
                nc.scalar.activation(
                    k4_all[:, t, :], qk_ps[:, CD:512], AF.Copy,
                    accum_out=stats_pack[:, t, 1, 0:1],
                )
                nc.vector.scalar_tensor_tensor(
                    junk, q4_all[:, t, :], 1.0, q4_all[:, t, :],
                    op0=ALU.mult, op1=ALU.mult,
                    accum_out=stats_pack[:, t, 0, 1:2],
                )
                nc.vector.scalar_tensor_tensor(
                    junk, k4_all[:, t, :], 1.0, k4_all[:, t, :],
                    op0=ALU.mult, op1=ALU.mult,
                    accum_out=stats_pack[:, t, 1, 1:2],
                )
                nc.scalar.activation(
                    v_blocks[:, t, :, 0:DH],
                    v_ps.rearrange("p (h d) -> p h d", h=HPC), AF.Copy,
                )

                # raw rope (LN affine folded in later), batched per 4 tiles
                if t % 4 == 3:
                    g = t - 3
                    if not use_qlw:
                        rope_emit(q4_all[:, g : t + 1, :], rot_q[:, g : t + 1, :],
                                  g, 4, "q")
                    if not use_klw:
                        rope_emit(k4_all[:, g : t + 1, :], rot_k[:, g : t + 1, :],
                                  g, 4, "k")

                for ci, (lo, hi) in enumerate(CH):
                    if t == hi - 1:
                        nc.gpsimd.dma_start(
                            ibs[ci],
                            stats_pack[:, lo:hi, :, :].rearrange("p t j s -> p (t j s)"),
                        )
                        nc.gpsimd.collective_compute(
                            "AllReduce", ALU.add, replica_groups=RG,
                            ins=[ibs[ci].opt()], outs=[obs[ci].opt()],
                        )
                        nc.sync.dma_start(
                            allred[:, lo:hi, :, :].rearrange("p t j s -> p (t j s)"),
                            obs[ci],
                        )
                if t == 12 and not (use_qlw or use_klw):
                    # hide the finalize of the first stats chunk under the last
                    # phase-1 tiles (fold math runs on the idle GpSimd engine)
                    finalizeA_state.append(finalize_stats(*CH[0]))
                    for ft in range(*CH[0]):
                        for j in range(2):
                            finalize_fold(ft, j, *finalizeA_state[0], CH[0][0])

        # ================= Phase 2: sparse attention =================
        with ExitStack() as phB:
            ps2 = phB.enter_context(tc.tile_pool(name="ps2", bufs=1, space="PSUM"))

            def attn_qb(qb):
                kts = kts_per_qb[qb]
                pairs = [kts[i : i + 2] for i in range(0, len(kts), 2)]
                npair = len(pairs)
                ctxT = p2.tile([P, 2, QB], BF16, tag="ctxT", bufs=2, name=f"ctxT{qb}")
                for hp in range(2):
                    ctx = ps2.tile([VB, 2, QB], F32, tag=f"ctx{hp}", bufs=1,
                                   name=f"ctx{qb}_{hp}")
                    for jj in range(2):
                        h = 2 * hp + jj
                        pend = None

                        def emit_pv(pi, pair, eT):
                            for i, kt in enumerate(pair):
                                nc.tensor.matmul(
                                    ctx[:, jj, :],
                                    v_sb[:, (kt * HPC + h) * VB : (kt * HPC + h) * VB + VB],
                                    eT[:, i, :],
                                    start=(pi == 0 and i == 0),
                                    stop=(pi == npair - 1 and i == len(pair) - 1),
                                )

                        for pi, pair in enumerate(pairs):
                            w = len(pair)
                            s_ps = ps2.tile([P, 2, QB], F32, tag="sc", bufs=3)
                            for i, kt in enumerate(pair):
                                nc.tensor.matmul(
                                    s_ps[:, i, :],
                                    kT[0:KROWS, h, kt * P : (kt + 1) * P],
                                    qT[0:KROWS, h, qb * QB : (qb + 1) * QB],
                                    start=True, stop=True,
                                )
                            eT = p2.tile([P, 2, QB], BF16, tag="eT", bufs=3)
                            nc.scalar.activation(eT[:, 0:w, :], s_ps[:, 0:w, :], AF.Exp)
                            if pend is not None:
                                emit_pv(*pend)
                            pend = (pi, pair, eT)
                        emit_pv(*pend)

                    # normalize: broadcast the denominator row to 64 rows by
                    # DMA (psum -> sbuf, stride-0 partition), then fast approx
                    # reciprocal (~18 bits, plenty for softmax)
                    z_sb = small.tile([1, 2, QB], F32, tag="z", name=f"z{qb}_{hp}")
                    nc.vector.tensor_copy(z_sb, ctx[DH : DH + 1, :, :])
                    zr_f = small.tile([1, 2, QB], F32, tag="zrf", name=f"zrf{qb}_{hp}")
                    nc.vector.reciprocal_approx_fast(zr_f, z_sb)
                    zr = small.tile([1, 2, QB], F32R, tag="zr", name=f"zr{qb}_{hp}")
                    nc.vector.tensor_copy(zr, zr_f)
                    repl = ps2.tile([DH, 2, QB], F32, tag="repl", bufs=1,
                                    name=f"repl{qb}_{hp}")
                    nc.tensor.matmul(
                        repl.rearrange("p a b -> p (a b)"), ones64,
                        zr.rearrange("p a b -> p (a b)"),
                        start=True, stop=True,
                    )
                    repl_sb = p2.tile([DH, 2, QB], F32, tag="replsb", bufs=2,
                                      name=f"replsb{qb}_{hp}")
                    nc.scalar.copy(repl_sb, repl)
                    for jj in range(2):
                        nc.vector.tensor_mul(
                            ctxT[jj * DH : (jj + 1) * DH, hp, :],
                            ctx[0:DH, jj, :],
                            repl_sb[:, jj, :],
                        )
                return ctxT

            def outproj_qb(qb, ctxT):
                # out projection for a q block (256 tokens = 2 out tiles)
                for tt in range(2):
                    gt = qb * 2 + tt
                    o_sb = p2.tile([P, D], BF16, tag="osb", bufs=2, name=f"osb{gt}")
                    for s in range(2):
                        o_ps = ps2.tile([P, 2, QB], F32, tag="sc", bufs=3,
                                        name=f"o{gt}_{s}")
                        ov = o_ps.rearrange("p a b -> p (a b)")
                        for c in range(2):
                            nc.tensor.matmul(
                                ov,
                                ctxT[:, c, tt * P : (tt + 1) * P],
                                wo_sb[:, c, s * 512 : (s + 1) * 512],
                                start=(c == 0), stop=(c == 1),
                            )
                        if s == 0:
                            nc.scalar.copy(o_sb[:, 0:512], ov)
                        else:
                            nc.vector.tensor_copy(o_sb[:, 512:1024], ov)
                    nc.sync.dma_start(out_t_d[gt], o_sb)

            # qbs grouped by the stats chunk covering their largest k-tile;
            # software-pipeline the out-projection one q-block behind attention
            # so PE never idles on the (DVE) normalize chain
            done = set()
            pending = None
            for ci, (lo, hi) in enumerate(CH):
                if ci > 0 or not finalizeA_state:
                    st = finalize_stats(lo, hi)
                    for ft in range(lo, hi):
                        for j in range(2):
                            finalize_fold(ft, j, *st, lo)
                for qb in range(NQB):
                    if qb in done or kts_per_qb[qb][-1] >= hi:
                        continue
                    done.add(qb)
                    ctxT = attn_qb(qb)
                    if pending is not None:
                        outproj_qb(*pending)
                    pending = (qb, ctxT)
            if pending is not None:
                outproj_qb(*pending)

    nc.compile()
    return nc


_CACHE = {}


def _get_nc(key):
    if key not in _CACHE:
        _CACHE[key] = build_bass(*key)
    return _CACHE[key]


def _plan(seq_id):
    """Compile-time sparsity plan from seq_id (union over both batches)."""
    kts_per_qb = []
    for qb in range(NQB):
        s = set()
        for b in range(B):
            sid = seq_id[b]
            segs = set(int(v) for v in sid[qb * QB : (qb + 1) * QB])
            for kt in range(TT):
                ksegs = set(int(v) for v in sid[kt * P : (kt + 1) * P])
                if ksegs & segs:
                    s.add(kt)
        kts_per_qb.append(tuple(sorted(s)))
    return (tuple(kts_per_qb),)


def _host_prep(x, seq_id, ln1_w, ln1_b, w_qkv, q_ln_w, k_ln_w, w_out):
    """Build the 8 per-core input maps + compile key."""
    x = np.asarray(x, np.float32)
    seq_id = np.asarray(seq_id)
    ln1_w = np.asarray(ln1_w, np.float32)
    ln1_b = np.asarray(ln1_b, np.float32)
    w_qkv = np.asarray(w_qkv, np.float32)
    q_ln_w = np.asarray(q_ln_w, np.float32)
    k_ln_w = np.asarray(k_ln_w, np.float32)
    w_out = np.asarray(w_out, np.float32)

    use_ln1b = bool(np.any(ln1_b != 0.0))
    use_qlw = not np.allclose(q_ln_w, 1.0)
    use_klw = not np.allclose(k_ln_w, 1.0)

    bf = ml_dtypes.bfloat16
    wq_f = w_qkv[:, 0:D] * ln1_w[:, None]
    wk_f = w_qkv[:, D : 2 * D] * ln1_w[:, None]
    wv_f = w_qkv[:, 2 * D : 3 * D] * ln1_w[:, None]

    # rope tables with 1/sqrt(sqrt(64)) on each side -> scores * 1/8;
    # replicated over the 4 local heads for flat 2D device ops
    inv_freq = 1.0 / (ROPE_BASE ** (np.arange(0, DH, 2, dtype=np.float32) / DH))
    tpos = np.arange(L, dtype=np.float32)
    freqs = np.einsum("l,f->lf", tpos, inv_freq)
    emb = np.concatenate([freqs, freqs], axis=-1)
    s8 = np.float32(8.0 ** -0.5)
    cos_t = (np.cos(emb) * s8).astype(np.float32)
    sin_t = (np.sin(emb) * s8).astype(np.float32)
    r1 = np.concatenate(
        [cos_t[:, : DH // 2] - sin_t[:, : DH // 2],
         cos_t[:, DH // 2 :] + sin_t[:, DH // 2 :]], axis=1
    )
    def pack(a):
        """[n*128, W] -> [128, n*W] partition-major (device tile [p, n, W])."""
        n = a.shape[0] // P
        return np.ascontiguousarray(
            a.reshape(n, P, a.shape[1]).transpose(1, 0, 2).reshape(P, -1)
        )

    cos4 = pack(np.tile(cos_t, (1, HPC))).astype(bf)
    sl4 = pack(np.tile(-sin_t[:, : DH // 2], (1, HPC))).astype(bf)
    sh4 = pack(np.tile(sin_t[:, DH // 2 :], (1, HPC))).astype(bf)
    r14 = pack(np.tile(r1, (1, HPC))).astype(bf)

    identb = np.eye(P, dtype=bf)
    ones64 = np.ones((1, DH), np.float32)

    (kts_per_qb,) = _plan(seq_id)
    key = (use_ln1b, use_qlw, use_klw, kts_per_qb)

    in_maps = []
    for c in range(8):
        b, g = c // HPC, c % HPC
        mine = np.arange(g * CD, (g + 1) * CD)

        sid = np.asarray(seq_id[b], np.int64)
        A = (sid[None, :] == np.arange(4)[:, None]).astype(np.float32)
        maskq = np.concatenate([MASK_A * A, MASK_A * np.ones((1, L), np.float32)])
        maskk = np.concatenate([MASK_A * A, -MASK_A * np.ones((1, L), np.float32)])

        m = {
            "x": np.ascontiguousarray(x[b]),
            "wqk": pack(
                np.concatenate([wq_f[:, mine], wk_f[:, mine]], axis=1)
            ).astype(bf),
            "wv": pack(wv_f[:, mine]).astype(bf),
            "wo": pack(w_out[mine, :]).astype(bf),
            "maskq": maskq.astype(bf),
            "maskk": maskk.astype(bf),
            "cos4": cos4,
            "sl4": sl4,
            "sh4": sh4,
            "r14": r14,
            "identb": identb,
            "ones64": ones64,
        }
        if use_ln1b:
            m["lnb"] = ln1_b.reshape(1, D)
        if use_qlw:
            m["qlw"] = q_ln_w[mine].reshape(1, CD)
        if use_klw:
            m["klw"] = k_ln_w[mine].reshape(1, CD)
        in_maps.append(m)
    return in_maps, key


def run(inputs, trace=False):
    """Run on hardware; returns (output [B, L, D] fp32, BassKernelResults)."""
    in_maps, key = _host_prep(**inputs)
    nc = _get_nc(key)
    res = bass_utils.run_bass_kernel_spmd(
        nc, in_maps, core_ids=list(range(8)), trace=trace
    )
    out = np.zeros((B, L, D), np.float32)
    for c in range(8):
        out[c // HPC] += np.asarray(res.results[c]["out"], dtype=np.float32)
    return out, res


def kernel(**inputs) -> np.ndarray:
    out, _ = run(inputs)
    return out


# revision 33
# speedup vs baseline: 1.1160x; 1.0198x over previous
"""Trainium2 Bass kernel for MHA block (LN -> QKV -> qk-LN -> RoPE -> masked attn -> out-proj).

Self-contained: hardcodes shapes B=2, L=2048, D=1024, H=16, Dh=64; runs on 8 NeuronCores
via bass_utils.run_bass_kernel_spmd. Sharding: core c = (batch b = c//4, head-group
g = c%4 of 4 heads). Weight columns are sliced per core so "our" 4 heads are always
columns 0:256 -> the device program is identical on all cores (SPMD).

Key structure (v2):
- bf16 compute throughout (weights, h, q/k/v, probs, ctx, out partials); fp32 PSUM.
- Sparse attention: seq_id is sorted per batch -> the mask is block diagonal. The
  kernel is compiled per seq-segment structure (computed from the actual input in
  kernel()); (q-block, k-tile) pairs with no segment overlap in EITHER batch are
  skipped entirely (scores, exp and PV). Masks rows (5 extra contraction rows at
  64:69 of qT/kT) give exact masking at segment boundaries.
- qk-LN stats (over the full 1024 dims) come from per-core partial sums gathered
  free on the ACT copy/square accumulators, with two 4-core-group AllReduces.
  Attention q-blocks are ordered so blocks only needing k-tiles < SP run while the
  second AllReduce is in flight.
- rstd = exp(-0.5*ln(var+eps)): Ln/Exp/Identity/Copy/Square all live in ONE ACT
  table -> no table-swap stalls between LN work and softmax exp.
- RoPE applied to raw q/k (linear), LN affine folded in afterwards:
  rot(LN(q)) = rstd*rot(q) + (-rstd*mu)*rot(ones); rope/fold are flat 2D bf16 ops
  with host-replicated tables.
"""

import numpy as np
import ml_dtypes
from contextlib import ExitStack

import concourse.bass as bass
import concourse.tile as tile
from concourse import bacc, mybir
from concourse import bass_utils

F32 = mybir.dt.float32
F32R = mybir.dt.float32r
BF16 = mybir.dt.bfloat16
AF = mybir.ActivationFunctionType
ALU = mybir.AluOpType

B, L, D = 2, 2048, 1024
H, DH = 16, 64
HPC = 4          # heads per core
CD = HPC * DH    # ctx dims per core = 256
P = 128
TT = L // P      # 16 token tiles
KC = D // P      # 8 contraction chunks
QB = 256         # query block width
NQB = L // QB    # 8 query blocks
EPS = 1e-5
ROPE_BASE = 10000.0
MASK_A = 8.0     # mask row scale; mask bias = -MASK_A^2 = -64 for masked pairs
KROWS = DH + 5   # contraction rows for scores: 64 dims + 5 mask rows
VB = DH + 1      # v block width (64 dims + ones col)
RG = [[0, 1, 2, 3], [4, 5, 6, 7]]


def build_bass(use_ln1b, use_qlw, use_klw, kts_per_qb):
    # stats-AllReduce chunks; a tiny barrier collective at kernel start
    # absorbs the inter-core launch skew so these land fast
    CH = [(0, 10), (10, 16)]
    nc = bacc.Bacc("TRN2", target_bir_lowering=False, debug=False, num_devices=8)

    # ---- DRAM I/O ----
    # weights/tables arrive host-prepacked partition-major ([128, contiguous])
    # so every DMA is 128 simple full lines: fast descriptor issue + bandwidth
    x_d = nc.dram_tensor("x", [L, D], F32, kind="ExternalInput").ap()
    wqk_d = nc.dram_tensor("wqk", [P, KC * 512], BF16, kind="ExternalInput").ap()
    wv_d = nc.dram_tensor("wv", [P, KC * CD], BF16, kind="ExternalInput").ap()
    wo_d = nc.dram_tensor("wo", [P, 2 * D], BF16, kind="ExternalInput").ap()
    mq_d = nc.dram_tensor("maskq", [5, L], BF16, kind="ExternalInput").ap()
    mk_d = nc.dram_tensor("maskk", [5, L], BF16, kind="ExternalInput").ap()
    cos4_d = nc.dram_tensor("cos4", [P, TT * CD], BF16, kind="ExternalInput").ap()
    sl4_d = nc.dram_tensor("sl4", [P, TT * CD // 2], BF16, kind="ExternalInput").ap()
    sh4_d = nc.dram_tensor("sh4", [P, TT * CD // 2], BF16, kind="ExternalInput").ap()
    r14_d = nc.dram_tensor("r14", [P, TT * CD], BF16, kind="ExternalInput").ap()
    idb_d = nc.dram_tensor("identb", [P, P], BF16, kind="ExternalInput").ap()
    on64_d = nc.dram_tensor("ones64", [1, DH], F32R, kind="ExternalInput").ap()
    if use_ln1b:
        lnb_d = nc.dram_tensor("lnb", [1, D], F32, kind="ExternalInput").ap()
    if use_qlw:
        qlw_d = nc.dram_tensor("qlw", [1, CD], F32, kind="ExternalInput").ap()
    if use_klw:
        klw_d = nc.dram_tensor("klw", [1, CD], F32, kind="ExternalInput").ap()
    out_d = nc.dram_tensor("out", [L, D], BF16, kind="ExternalOutput").ap()

    x_t_d = x_d.rearrange("(n p) d -> n p d", p=P)
    out_t_d = out_d.rearrange("(n p) d -> n p d", p=P)


    with tile.TileContext(nc) as tc, ExitStack() as ctx:
        cpool = ctx.enter_context(tc.tile_pool(name="cpool", bufs=1))
        small = ctx.enter_context(tc.tile_pool(name="small", bufs=4))
        pstp = ctx.enter_context(tc.tile_pool(name="pstp", bufs=1, space="PSUM"))
        dramp = ctx.enter_context(tc.tile_pool(name="dramp", bufs=1, space="DRAM"))

        # --- persistent SBUF ---
        # DMA issue time is serial per queue (~0.7us each): spread the initial
        # loads across engine queues so issue parallelizes and x tiles (on
        # sync) aren't stuck behind weight/table loads.
        identb = cpool.tile([P, P], BF16)
        nc.gpsimd.dma_start(identb, idb_d)
        eps_ap = cpool.tile([P, 1], F32)
        nc.vector.memset(eps_ap, EPS)
        ones64 = cpool.tile([1, DH], F32R)
        nc.gpsimd.dma_start(ones64, on64_d)

        wqk_sb = cpool.tile([P, KC, 512], BF16)
        wqk_f = wqk_sb.rearrange("p c n -> p (c n)")
        for i in range(4):
            sl = slice(i * KC * 128, (i + 1) * KC * 128)
            nc.scalar.dma_start(wqk_f[:, sl], wqk_d[:, sl])
        wv_sb = cpool.tile([P, KC, CD], BF16)
        wv_f = wv_sb.rearrange("p c n -> p (c n)")
        for i in range(2):
            sl = slice(i * KC * P, (i + 1) * KC * P)
            nc.scalar.dma_start(wv_f[:, sl], wv_d[:, sl])

        cos4_sb = cpool.tile([P, TT, CD], BF16)
        nc.gpsimd.dma_start(cos4_sb.rearrange("p t d -> p (t d)"), cos4_d)
        sl4_sb = cpool.tile([P, TT, CD // 2], BF16)
        nc.gpsimd.dma_start(sl4_sb.rearrange("p t d -> p (t d)"), sl4_d)
        sh4_sb = cpool.tile([P, TT, CD // 2], BF16)
        nc.gpsimd.dma_start(sh4_sb.rearrange("p t d -> p (t d)"), sh4_d)

        # qT/kT: rows 0:64 head dims (transposed), 64:69 mask rows; rows 69:127
        # never read (scores contract only 0:69) -> no zero fill needed.
        qT = cpool.tile([P, HPC, L], BF16)
        kT = cpool.tile([P, HPC, L], BF16)
        for hh in range(HPC):
            nc.gpsimd.dma_start(qT[DH:KROWS, hh, :], mq_d)
            nc.gpsimd.dma_start(kT[DH:KROWS, hh, :], mk_d)

        # v blocks: [128 keys, kt, h, 64 dims + ones col]
        v_sb = cpool.tile([P, TT * HPC * VB], BF16)
        v_blocks = v_sb.rearrange("p (t h d) -> p t h d", t=TT, h=HPC)
        nc.gpsimd.memset(v_blocks[:, :, :, DH : DH + 1], 1.0)

        r14_sb = cpool.tile([P, TT, CD], BF16)
        nc.gpsimd.dma_start(r14_sb.rearrange("p t d -> p (t d)"), r14_d)

        wo_sb = cpool.tile([P, CD // P, D], BF16)
        nc.gpsimd.dma_start(wo_sb.rearrange("p c n -> p (c n)"), wo_d)

        if use_ln1b:
            lnb_sb = cpool.tile([P, D], F32)
            nc.sync.dma_start(lnb_sb, lnb_d.partition_broadcast(P)[:, 0, :])
        if use_qlw:
            qlw_sb = cpool.tile([P, CD], F32)
            nc.sync.dma_start(qlw_sb, qlw_d.partition_broadcast(P)[:, 0, :])
        if use_klw:
            klw_sb = cpool.tile([P, CD], F32)
            nc.sync.dma_start(klw_sb, klw_d.partition_broadcast(P)[:, 0, :])

        q4_all = cpool.tile([P, TT, CD], BF16)
        k4_all = cpool.tile([P, TT, CD], BF16)
        rot_q = cpool.tile([P, TT, CD], BF16)
        rot_k = cpool.tile([P, TT, CD], BF16)
        stats_pack = cpool.tile([P, TT, 2, 2], F32)   # [s1, s2] per (t, q/k)
        allred = cpool.tile([P, TT, 2, 2], F32)
        junk = cpool.tile([P, CD], BF16)

        ibs = [dramp.tile([P, (hi - lo) * 4], F32, name=f"ib{i}")
               for i, (lo, hi) in enumerate(CH)]
        obs = [dramp.tile([P, (hi - lo) * 4], F32, name=f"ob{i}")
               for i, (lo, hi) in enumerate(CH)]

        p2 = ctx.enter_context(tc.tile_pool(name="p2", bufs=2))

        def rope_emit(src_ap, dst_ap, lo, nt, tag):
            """dst = src*cos + rothalf(src)*sin for `nt` token tiles at once
            (flat bf16 ops, batched to amortize per-op overhead; sin signed)."""
            qa = p2.tile([P, 4, CD], BF16, tag=f"qa{tag}", bufs=2)
            nc.vector.tensor_mul(qa[:, 0:nt, :], src_ap, cos4_sb[:, lo : lo + nt, :])
            qbt = p2.tile([P, 4, HPC, 2, DH // 2], BF16, tag=f"qb{tag}", bufs=2)
            srcv = src_ap.rearrange("p t (h s d) -> p t h s d", h=HPC, s=2)
            nc.gpsimd.tensor_mul(
                qbt[:, 0:nt, :, 0, :], srcv[:, :, :, 1, :],
                sl4_sb[:, lo : lo + nt, :].rearrange("p t (h d) -> p t h d", h=HPC),
            )
            nc.gpsimd.tensor_mul(
                qbt[:, 0:nt, :, 1, :], srcv[:, :, :, 0, :],
                sh4_sb[:, lo : lo + nt, :].rearrange("p t (h d) -> p t h d", h=HPC),
            )
            nc.vector.tensor_add(
                dst_ap, qa[:, 0:nt, :],
                qbt[:, 0:nt, :, :, :].rearrange("p t h s d -> p t (h s d)"),
            )

        def finalize_stats(lo, hi):
            """qk-LN mu/rstd from the all-reduced sums for tiles [lo, hi)."""
            n = hi - lo
            mu = small.tile([P, n, 2], F32, tag="fmu", name=f"fmu{lo}")
            nc.vector.tensor_scalar(mu, allred[:, lo:hi, :, 0], 1.0 / D, None, ALU.mult)
            m2 = small.tile([P, n, 2], F32, tag="fm2", name=f"fm2{lo}")
            nc.vector.tensor_mul(m2, mu, mu)
            var = small.tile([P, n, 2], F32, tag="fvar", name=f"fvar{lo}")
            nc.vector.scalar_tensor_tensor(
                var, allred[:, lo:hi, :, 1], 1.0 / D, m2,
                op0=ALU.mult, op1=ALU.subtract,
            )
            rstd = small.tile([P, n, 2], F32, tag="frstd", name=f"frstd{lo}")
            nc.scalar.activation(rstd, var, AF.Sqrt, bias=eps_ap)
            nc.vector.reciprocal(rstd, rstd)
            nm = small.tile([P, n, 2], F32, tag="fnm", name=f"fnm{lo}")
            nc.vector.scalar_tensor_tensor(nm, mu, -1.0, rstd, op0=ALU.mult, op1=ALU.mult)
            return rstd, nm

        def finalize_fold(t, j, rstd, nm, lo):
            """Fold LN affine into rope'd q/k for tile t, build qT/kT columns."""
            rs = rstd[:, t - lo, j : j + 1]
            nmj = nm[:, t - lo, j : j + 1]
            lw_flag = use_qlw if j == 0 else use_klw
            foldt = p2.tile([P, CD], BF16, tag="fold", bufs=3)
            if lw_flag:
                src4 = q4_all if j == 0 else k4_all
                lw_sb = qlw_sb if j == 0 else klw_sb
                xn = p2.tile([P, 1, CD], BF16, tag="xn", bufs=2)
                nc.scalar.activation(
                    xn[:, 0, :], src4[:, t, :], AF.Identity, bias=nmj, scale=rs
                )
                nc.vector.tensor_mul(xn[:, 0, :], xn[:, 0, :], lw_sb)
                rope_emit(xn, foldt.rearrange("p (t d) -> p t d", t=1), t, 1, "f")
            else:
                rot = rot_q if j == 0 else rot_k
                tmp = p2.tile([P, CD], BF16, tag="ftmp", bufs=2)
                nmb = bass.AP(tensor=nmj.tensor, offset=nmj.offset,
                              ap=[nmj.ap[0], [0, CD]])
                nc.gpsimd.tensor_mul(tmp, r14_sb[:, t, :], nmb)
                nc.vector.scalar_tensor_tensor(
                    foldt, rot[:, t, :], rs, tmp, op0=ALU.mult, op1=ALU.add
                )
            tp = pstp.tile([DH, HPC, P], BF16, tag="tp", bufs=2)
            for hh in range(HPC):
                nc.tensor.transpose(
                    tp[:, hh, :], foldt[:, hh * DH : (hh + 1) * DH], identb
                )
            dst = qT if j == 0 else kT
            nc.vector.tensor_copy(dst[0:DH, :, t * P : (t + 1) * P], tp)

        finalizeA_state = []

        # ================= Phase 1: LN1 + QKV + raw rope + stats =================
        with ExitStack() as phA:
            p1 = phA.enter_context(tc.tile_pool(name="p1", bufs=2))
            psA = phA.enter_context(tc.tile_pool(name="psA", bufs=1, space="PSUM"))

            xtiles = {}

            def xfetch(t):
                if t >= TT or t in xtiles:
                    return
                xt = p1.tile([P, D], F32, tag="x", bufs=4, name=f"x{t}")
                nc.sync.dma_start(xt[:, 0:512], x_t_d[t][:, 0:512])
                nc.sync.dma_start(xt[:, 512:1024], x_t_d[t][:, 512:1024])
                xtiles[t] = xt

            for t in range(3):
                xfetch(t)

            for t in range(TT):
                xfetch(t + 3)
                x_t = xtiles.pop(t)

                # LN1 stats
                xstats = small.tile([P, 2, 6], F32, tag="xstats")
                for s in range(2):
                    nc.vector.bn_stats(
                        xstats[:, s, :],
                        x_t[:, s * 512 : (s + 1) * 512].rearrange(
                            "p (s d) -> p s d", s=1
                        ),
                    )
                xmv = small.tile([P, 2], F32, tag="xmv")
                nc.vector.bn_aggr(xmv, xstats)
                xrstd = small.tile([P, 1], F32, tag="xrstd")
                nc.scalar.activation(xrstd, xmv[:, 1:2], AF.Sqrt, bias=eps_ap)
                nc.vector.reciprocal(xrstd, xrstd)
                xnm = small.tile([P, 1], F32, tag="xnm")
                nc.vector.tensor_scalar(xnm, xmv[:, 0:1], xrstd, -1.0, ALU.mult, ALU.mult)
                h_t = p1.tile([P, D], BF16, tag="h", bufs=2)
                nc.scalar.activation(h_t, x_t, AF.Identity, bias=xnm, scale=xrstd)
                if use_ln1b:
                    nc.vector.tensor_add(h_t, h_t, lnb_sb)

                # hT (bf16 transposes)
                ht_ps = psA.tile([P, KC, P], BF16, tag="ht", bufs=2)
                for c in range(KC):
                    nc.tensor.transpose(ht_ps[:, c, :], h_t[:, c * P : (c + 1) * P], identb)
                hT_t = p1.tile([P, KC, P], BF16, tag="hT", bufs=2)
                nc.vector.tensor_copy(hT_t, ht_ps)

                # QKV
                qk_ps = psA.tile([P, 512], F32, tag="qk", bufs=2)
                v_ps = psA.tile([P, CD], F32, tag="v", bufs=2)
                for c in range(KC):
                    nc.tensor.matmul(qk_ps, hT_t[:, c, :], wqk_sb[:, c, :],
                                     start=(c == 0), stop=(c == KC - 1))
                    nc.tensor.matmul(v_ps, hT_t[:, c, :], wv_sb[:, c, :],
                                     start=(c == 0), stop=(c == KC - 1))

                # psum -> sbuf copies; s1/s2 accumulate for free on ACT
                nc.scalar.activation(
                    q4_all[:, t, :], qk_ps[:, 0:CD], AF.Copy,
                    accum_out=stats_pack[:, t, 0, 0:1],
                )
                nc.scalar.activation(
                    k4_all[:, t, :], qk_ps[:, CD:512], AF.Copy,
                    accum_out=stats_pack[:, t, 1, 0:1],
                )
                nc.vector.scalar_tensor_tensor(
                    junk, q4_all[:, t, :], 1.0, q4_all[:, t, :],
                    op0=ALU.mult, op1=ALU.mult,
                    accum_out=stats_pack[:, t, 0, 1:2],
                )
                nc.vector.scalar_tensor_tensor(
                    junk, k4_all[:, t, :], 1.0, k4_all[:, t, :],
                    op0=ALU.mult, op1=ALU.mult,
                    accum_out=stats_pack[:, t, 1, 1:2],
                )
                nc.scalar.activation(
                    v_blocks[:, t, :, 0:DH],
                    v_ps.rearrange("p (h d) -> p h d", h=HPC), AF.Copy,
                )

                # raw rope (LN affine folded in later), batched per 4 tiles
                if t % 4 == 3:
                    g = t - 3
                    if not use_qlw:
                        rope_emit(q4_all[:, g : t + 1, :], rot_q[:, g : t + 1, :],
                                  g, 4, "q")
                    if not use_klw:
                        rope_emit(k4_all[:, g : t + 1, :], rot_k[:, g : t + 1, :],
                                  g, 4, "k")

                for ci, (lo, hi) in enumerate(CH):
                    if t == hi - 1:
                        nc.gpsimd.dma_start(
                            ibs[ci],
                            stats_pack[:, lo:hi, :, :].rearrange("p t j s -> p (t j s)"),
                        )
                        nc.gpsimd.collective_compute(
                            "AllReduce", ALU.add, replica_groups=RG,
                            ins=[ibs[ci].opt()], outs=[obs[ci].opt()],
                        )
                        nc.sync.dma_start(
                            allred[:, lo:hi, :, :].rearrange("p t j s -> p (t j s)"),
                            obs[ci],
                        )
                if t == 13 and not (use_qlw or use_klw):
                    # hide the finalize of the first stats chunk under the last
                    # phase-1 tiles (fold math runs on the idle GpSimd engine)
                    finalizeA_state.append(finalize_stats(*CH[0]))
                    for ft in range(*CH[0]):
                        for j in range(2):
                            finalize_fold(ft, j, *finalizeA_state[0], CH[0][0])

        # ================= Phase 2: sparse attention =================
        with ExitStack() as phB:
            ps2 = phB.enter_context(tc.tile_pool(name="ps2", bufs=1, space="PSUM"))

            def attn_emit(qb, ctxs, pairs, plo, phi):
                # each attn_emit call is a CLOSED psum accumulation group
                # (start only on the very first PV, stop on this part's last;
                # a later part re-opens with start=False and accumulates on)
                npair = len(pairs)
                for hp in range(2):
                    ctx = ctxs[hp]
                    for jj in range(2):
                        h = 2 * hp + jj
                        pend = None

                        def emit_pv(pi, pair, eT):
                            for i, kt in enumerate(pair):
                                nc.tensor.matmul(
                                    ctx[:, jj, :],
                                    v_sb[:, (kt * HPC + h) * VB : (kt * HPC + h) * VB + VB],
                                    eT[:, i, :],
                                    start=(pi == 0 and i == 0),
                                    stop=(pi == phi - 1 and i == len(pair) - 1),
                                    skip_group_check=True,
                                )

                        for pi in range(plo, phi):
                            pair = pairs[pi]
                            w = len(pair)
                            s_ps = ps2.tile([P, 2, QB], F32, tag="sc", bufs=3)
                            for i, kt in enumerate(pair):
                                nc.tensor.matmul(
                                    s_ps[:, i, :],
                                    kT[0:KROWS, h, kt * P : (kt + 1) * P],
                                    qT[0:KROWS, h, qb * QB : (qb + 1) * QB],
                                    start=True, stop=True,
                                )
                            eT = p2.tile([P, 2, QB], BF16, tag="eT", bufs=3)
                            nc.scalar.activation(eT[:, 0:w, :], s_ps[:, 0:w, :], AF.Exp)
                            if pend is not None:
                                emit_pv(*pend)
                            pend = (pi, pair, eT)
                        if pend is not None:
                            emit_pv(*pend)

            def attn_qb_start(qb, thresh):
                kts = kts_per_qb[qb]
                pairs = [kts[i : i + 2] for i in range(0, len(kts), 2)]
                nearly = sum(1 for p in pairs if max(p) < thresh)
                ctxT = p2.tile([P, 2, QB], BF16, tag="ctxT", bufs=2, name=f"ctxT{qb}")
                ctxs = [ps2.tile([VB, 2, QB], F32, tag=f"ctx{hp}", bufs=1,
                                 name=f"ctx{qb}_{hp}") for hp in range(2)]
                attn_emit(qb, ctxs, pairs, 0, nearly)
                return (qb, pairs, nearly, ctxs, ctxT)

            def attn_qb_finish(state):
                qb, pairs, nearly, ctxs, ctxT = state
                attn_emit(qb, ctxs, pairs, nearly, len(pairs))
                for hp in range(2):
                    ctx = ctxs[hp]
                    # normalize: 1/denominator row (fast approx, ~18 bits),
                    # replicated to 64 rows in one K=1 matmul
                    z_sb = small.tile([1, 2, QB], F32, tag="z", name=f"z{qb}_{hp}")
                    nc.vector.tensor_copy(z_sb, ctx[DH : DH + 1, :, :])
                    zr_f = small.tile([1, 2, QB], F32, tag="zrf", name=f"zrf{qb}_{hp}")
                    nc.vector.reciprocal_approx_fast(zr_f, z_sb)
                    zr = small.tile([1, 2, QB], F32R, tag="zr", name=f"zr{qb}_{hp}")
                    nc.vector.tensor_copy(zr, zr_f)
                    repl = ps2.tile([DH, 2, QB], F32, tag="repl", bufs=1,
                                    name=f"repl{qb}_{hp}")
                    nc.tensor.matmul(
                        repl.rearrange("p a b -> p (a b)"), ones64,
                        zr.rearrange("p a b -> p (a b)"),
                        start=True, stop=True,
                    )
                    repl_sb = p2.tile([DH, 2, QB], F32, tag="replsb", bufs=2,
                                      name=f"replsb{qb}_{hp}")
                    nc.scalar.copy(repl_sb, repl)
                    for jj in range(2):
                        nc.vector.tensor_mul(
                            ctxT[jj * DH : (jj + 1) * DH, hp, :],
                            ctx[0:DH, jj, :],
                            repl_sb[:, jj, :],
                        )
                return ctxT

            def attn_qb(qb):
                return attn_qb_finish(attn_qb_start(qb, TT + 1))

            def outproj_qb(qb, ctxT):
                # out projection for a q block (256 tokens = 2 out tiles)
                for tt in range(2):
                    gt = qb * 2 + tt
                    o_sb = p2.tile([P, D], BF16, tag="osb", bufs=2, name=f"osb{gt}")
                    for s in range(2):
                        o_ps = ps2.tile([P, 2, QB], F32, tag="sc", bufs=3,
                                        name=f"o{gt}_{s}")
                        ov = o_ps.rearrange("p a b -> p (a b)")
                        for c in range(2):
                            nc.tensor.matmul(
                                ov,
                                ctxT[:, c, tt * P : (tt + 1) * P],
                                wo_sb[:, c, s * 512 : (s + 1) * 512],
                                start=(c == 0), stop=(c == 1),
                            )
                        if s == 0:
                            nc.scalar.copy(o_sb[:, 0:512], ov)
                        else:
                            nc.vector.tensor_copy(o_sb[:, 512:1024], ov)
                    nc.sync.dma_start(out_t_d[gt], o_sb)

            # qbs grouped by the stats chunk covering their largest k-tile;
            # software-pipeline the out-projection one q-block behind attention.
            # The first chunk-2 qb whose own q-tiles sit in chunk 1 has its
            # chunk-1 scores emitted BEFORE the chunk-2 finalize, so the PE
            # queue has ready work while the second AllReduce completes.
            S1 = CH[0][1]
            group1 = [qb for qb in range(NQB) if kts_per_qb[qb][-1] < S1]
            group2 = [qb for qb in range(NQB) if kts_per_qb[qb][-1] >= S1]
            group2.sort(key=lambda q: -sum(1 for kt in kts_per_qb[q] if kt < S1)
                        if 2 * q + 1 < S1 else 0)
            split_qb = None  # split-across-finalize disabled: corrupts the split qb

            pending = None
            if not finalizeA_state:
                st = finalize_stats(*CH[0])
                for ft in range(*CH[0]):
                    for j in range(2):
                        finalize_fold(ft, j, *st, CH[0][0])
            for qb in group1:
                ctxT = attn_qb(qb)
                if pending is not None:
                    outproj_qb(*pending)
                pending = (qb, ctxT)
            pre = attn_qb_start(split_qb, S1) if split_qb is not None else None
            st2 = finalize_stats(*CH[1])
            for ft in range(*CH[1]):
                for j in range(2):
                    finalize_fold(ft, j, *st2, CH[1][0])
            if pre is not None:
                ctxT = attn_qb_finish(pre)
                if pending is not None:
                    outproj_qb(*pending)
                pending = (split_qb, ctxT)
            for qb in group2:
                if qb == split_qb:
                    continue
                ctxT = attn_qb(qb)
                if pending is not None:
                    outproj_qb(*pending)
                pending = (qb, ctxT)
            if pending is not None:
                outproj_qb(*pending)

    nc.compile()
    return nc


_CACHE = {}


def _get_nc(key):
    if key not in _CACHE:
        _CACHE[key] = build_bass(*key)
    return _CACHE[key]


def _plan(seq_id):
    """Compile-time sparsity plan from seq_id (union over both batches)."""
    kts_per_qb = []
    for qb in range(NQB):
        s = set()
        for b in range(B):
            sid = seq_id[b]
            segs = set(int(v) for v in sid[qb * QB : (qb + 1) * QB])
            for kt in range(TT):
                ksegs = set(int(v) for v in sid[kt * P : (kt + 1) * P])
                if ksegs & segs:
                    s.add(kt)
        kts_per_qb.append(tuple(sorted(s)))
    return (tuple(kts_per_qb),)


def _host_prep(x, seq_id, ln1_w, ln1_b, w_qkv, q_ln_w, k_ln_w, w_out):
    """Build the 8 per-core input maps + compile key."""
    x = np.asarray(x, np.float32)
    seq_id = np.asarray(seq_id)
    ln1_w = np.asarray(ln1_w, np.float32)
    ln1_b = np.asarray(ln1_b, np.float32)
    w_qkv = np.asarray(w_qkv, np.float32)
    q_ln_w = np.asarray(q_ln_w, np.float32)
    k_ln_w = np.asarray(k_ln_w, np.float32)
    w_out = np.asarray(w_out, np.float32)

    use_ln1b = bool(np.any(ln1_b != 0.0))
    use_qlw = not np.allclose(q_ln_w, 1.0)
    use_klw = not np.allclose(k_ln_w, 1.0)

    bf = ml_dtypes.bfloat16
    wq_f = w_qkv[:, 0:D] * ln1_w[:, None]
    wk_f = w_qkv[:, D : 2 * D] * ln1_w[:, None]
    wv_f = w_qkv[:, 2 * D : 3 * D] * ln1_w[:, None]

    # rope tables with 1/sqrt(sqrt(64)) on each side -> scores * 1/8;
    # replicated over the 4 local heads for flat 2D device ops
    inv_freq = 1.0 / (ROPE_BASE ** (np.arange(0, DH, 2, dtype=np.float32) / DH))
    tpos = np.arange(L, dtype=np.float32)
    freqs = np.einsum("l,f->lf", tpos, inv_freq)
    emb = np.concatenate([freqs, freqs], axis=-1)
    s8 = np.float32(8.0 ** -0.5)
    cos_t = (np.cos(emb) * s8).astype(np.float32)
    sin_t = (np.sin(emb) * s8).astype(np.float32)
    r1 = np.concatenate(
        [cos_t[:, : DH // 2] - sin_t[:, : DH // 2],
         cos_t[:, DH // 2 :] + sin_t[:, DH // 2 :]], axis=1
    )
    def pack(a):
        """[n*128, W] -> [128, n*W] partition-major (device tile [p, n, W])."""
        n = a.shape[0] // P
        return np.ascontiguousarray(
            a.reshape(n, P, a.shape[1]).transpose(1, 0, 2).reshape(P, -1)
        )

    cos4 = pack(np.tile(cos_t, (1, HPC))).astype(bf)
    sl4 = pack(np.tile(-sin_t[:, : DH // 2], (1, HPC))).astype(bf)
    sh4 = pack(np.tile(sin_t[:, DH // 2 :], (1, HPC))).astype(bf)
    r14 = pack(np.tile(r1, (1, HPC))).astype(bf)

    identb = np.eye(P, dtype=bf)
    ones64 = np.ones((1, DH), np.float32)

    (kts_per_qb,) = _plan(seq_id)
    key = (use_ln1b, use_qlw, use_klw, kts_per_qb)

    in_maps = []
    for c in range(8):
        b, g = c // HPC, c % HPC
        mine = np.arange(g * CD, (g + 1) * CD)

        sid = np.asarray(seq_id[b], np.int64)
        A = (sid[None, :] == np.arange(4)[:, None]).astype(np.float32)
        maskq = np.concatenate([MASK_A * A, MASK_A * np.ones((1, L), np.float32)])
        maskk = np.concatenate([MASK_A * A, -MASK_A * np.ones((1, L), np.float32)])

        m = {
            "x": np.ascontiguousarray(x[b]),
            "wqk": pack(
                np.concatenate([wq_f[:, mine], wk_f[:, mine]], axis=1)
            ).astype(bf),
            "wv": pack(wv_f[:, mine]).astype(bf),
            "wo": pack(w_out[mine, :]).astype(bf),
            "maskq": maskq.astype(bf),
            "maskk": maskk.astype(bf),
            "cos4": cos4,
            "sl4": sl4,
            "sh4": sh4,
            "r14": r14,
            "identb": identb,
            "ones64": ones64,
        }
        if use_ln1b:
            m["lnb"] = ln1_b.reshape(1, D)
        if use_qlw:
            m["qlw"] = q_ln_w[mine].reshape(1, CD)
        if use_klw:
            m["klw"] = k_ln_w[mine].reshape(1, CD)
        in_maps.append(m)
    return in_maps, key


def run(inputs, trace=False):
    """Run on hardware; returns (output [B, L, D] fp32, BassKernelResults)."""
    in_maps, key = _host_prep(**inputs)
    nc = _get_nc(key)
    res = bass_utils.run_bass_kernel_spmd(
        nc, in_maps, core_ids=list(range(8)), trace=trace
    )
    out = np.zeros((B, L, D), np.float32)
    for c in range(8):
        out[c // HPC] += np.asarray(res.results[c]["out"], dtype=np.float32)
    return out, res


def kernel(**inputs) -> np.ndarray:
    out, _ = run(inputs)
    return out
